# revision 3
# baseline (speedup 1.0000x reference)
"""IterSpatialCorrelationSampler (P=9, DP=1) Trainium2 Bass kernel.

out[b,i,j,y,x] = sum_c in1[b,c,y,x] * pad(in2)[b,c,y+i,x+j]   (pad=4 each side)

Strategy (v2 — ly-group band stores):
  - 8 cores, each handles (b, yhalf): b = core//2, 48 rows of y.
  - TensorE Gram-band formulation with a 16y x 8x m-tile (128 output
    positions on PSUM partitions, p = ly*8+lx) against a 24y x 16x window
    of padded in2 (n = 384 free), contraction over c (2 accumulating
    k=128 matmuls).
  - Key observation: for partition p = (ly, lx) the 81 useful band values
    sit at band[p, ly*16 + (di*16 + lx + dj)].  Within one ly-group
    (8 partitions, same ly) the useful span [ly*16, ly*16+144) is uniform,
    so a plain rectangular DMA band[8ly:8ly+8, t0:t1, 16ly:16ly+144]
    ships only 144 of 384 band elements per position: 1.77 MB/core
    instead of 4.72 MB.  One store per (ly, ty-row) = 48 stores (last
    row split in half to shorten the drain tail).
  - Inputs quantized host-side to fp8 e3m4 (rel err 1.8e-2, under the
    2e-2 bar); PSUM accumulates fp32; band ships as f16.
  - PSUM→SBUF copies alternate DVE/ACT; loads are emitted in deadline
    order across the two HWDGE queues; ty0 runs a ch0-only pass over its
    first 8 tiles so the PE has work before the ch1 image chunk lands;
    6 dummy matmuls during the load window ramp the PE clock (HAM).
"""

import numpy as np

import concourse.bass as bass
import concourse.bacc as bacc
import concourse.tile as tile
import concourse.mybir as mybir
from concourse.bass_utils import run_bass_kernel_spmd

# problem constants (hardcoded per contract)
B, C, H, W = 4, 256, 96, 128
P = 9
OFF = 4
NCORES = 8
YH = H // 2          # 48 rows per core
WP = W + 2 * OFF     # 136
ROWS = YH + 2 * OFF  # 56 rows of padded in2 per core
MT_Y, MT_X = 16, 8   # m-tile shape (16y x 8x = 128 partitions)
NW_Y, NW_X = MT_Y + P - 1, MT_X + P - 1   # 24 x 16 window
NTY, NTX = YH // MT_Y, W // MT_X          # 3 x 16 = 48 tiles
NT = NTY * NTX
NFREE = NW_Y * NW_X                       # 384
SPAN = P * NW_X                           # 144 elems shipped per position

_F8 = mybir.dt.np(mybir.dt.float8e3)   # ml_dtypes.float8_e3m4

_cached = {}


def _build():
    nc = bacc.Bacc(
        "TRN2",
        target_bir_lowering=False,
        debug=False,
        enable_asserts=False,
        num_devices=NCORES,
    )
    f16 = mybir.dt.float16
    f32 = mybir.dt.float32
    f8 = mybir.dt.float8e3

    in1_d = nc.dram_tensor("in1t", [128, NT, 2, MT_Y * MT_X], f8, kind="ExternalInput").ap()
    in2_d = nc.dram_tensor("in2c", [128, 2, ROWS, WP], f8, kind="ExternalInput").ap()
    band_d = nc.dram_tensor("band", [128, NT, SPAN], f16, kind="ExternalOutput").ap()

    with tile.TileContext(nc) as tc:
        with (
            tc.tile_pool(name="sb", bufs=1) as sb,
            tc.tile_pool(name="ps", bufs=8, space="PSUM") as ps,
        ):
            in2_sb = sb.tile([128, 2, ROWS, WP], f8, name="in2sb")
            in1_sb = sb.tile([128, NT, 2, MT_Y * MT_X], f8, name="in1sb")
            band = sb.tile([128, NT, NFREE], f16, name="band")

            # Loads in deadline order, alternating the two HWDGE queues.
            nc.sync.dma_start(out=in2_sb[:, 0, 0:24, :], in_=in2_d[:, 0, 0:24, :])
            nc.scalar.dma_start(out=in1_sb[:, 0:8, :, :], in_=in1_d[:, 0:8, :, :])
            nc.sync.dma_start(out=in2_sb[:, 1, 0:24, :], in_=in2_d[:, 1, 0:24, :])
            nc.scalar.dma_start(out=in1_sb[:, 8:16, :, :], in_=in1_d[:, 8:16, :, :])
            nc.sync.dma_start(out=in2_sb[:, 0, 24:40, :], in_=in2_d[:, 0, 24:40, :])
            nc.scalar.dma_start(out=in2_sb[:, 1, 24:40, :], in_=in2_d[:, 1, 24:40, :])
            nc.sync.dma_start(out=in1_sb[:, 16:32, :, :], in_=in1_d[:, 16:32, :, :])
            nc.scalar.dma_start(out=in2_sb[:, 0, 40:ROWS, :], in_=in2_d[:, 0, 40:ROWS, :])
            nc.sync.dma_start(out=in2_sb[:, 1, 40:ROWS, :], in_=in2_d[:, 1, 40:ROWS, :])
            nc.scalar.dma_start(out=in1_sb[:, 32:NT, :, :], in_=in1_d[:, 32:NT, :, :])

            # Warm up the PE while the first loads stream in (HAM clock
            # ramp 1.2 -> 2.4 GHz needs ~3.4us of sustained PE activity).
            wu = sb.tile([128, 512], f8, name="wu")
            nc.vector.memset(wu[:, :], 0.0)
            wpt = ps.tile([128, 512], f32, tag="pt", name="wpt")
            for _ in range(6):
                nc.tensor.matmul(wpt[:, :], wu[:, 0:128], wu[:, :], start=True, stop=True)

            def win_ap(ch, ty, tx):
                return in2_sb[
                    :, ch,
                    MT_Y * ty : MT_Y * ty + NW_Y,
                    MT_X * tx : MT_X * tx + NW_X,
                ]

            def finish(t, pt):
                # PSUM -> SBUF f16 copy, alternating engines.
                if t % 2 == 0:
                    nc.vector.tensor_copy(band[:, t, :], pt[:, 0:NFREE])
                else:
                    nc.scalar.mul(band[:, t, :], pt[:, 0:NFREE], 1.0)

            qi = 0

            def store_rows(t0, t1):
                # One DMA per ly-group: partitions 8ly..8ly+8 share the
                # useful span [16ly, 16ly+144) in every tile's band.
                nonlocal qi
                for ly in range(MT_Y):
                    src = band[8 * ly : 8 * ly + 8, t0:t1, 16 * ly : 16 * ly + SPAN]
                    dst = band_d[8 * ly : 8 * ly + 8, t0:t1, :]
                    eng = nc.sync if qi == 0 else nc.scalar
                    qi ^= 1
                    eng.dma_start(out=dst, in_=src)

            for ty in range(NTY):
                if ty == 0:
                    # ch0-only pass over tx 0..7 (ch1 image lands later)
                    pts = []
                    for tx in range(8):
                        pt = ps.tile([128, 512], f32, tag="pt", name=f"pt0_{tx}")
                        pts.append(pt)
                        nc.tensor.matmul(
                            pt[:, 0:NFREE], in1_sb[:, tx, 0, :], win_ap(0, 0, tx),
                            start=True, stop=False,
                        )
                    for tx in range(8):
                        nc.tensor.matmul(
                            pts[tx][:, 0:NFREE], in1_sb[:, tx, 1, :], win_ap(1, 0, tx),
                            start=False, stop=True,
                        )
                        finish(tx, pts[tx])
                    rng = range(8, NTX)
                else:
                    rng = range(NTX)
                for tx in rng:
                    t = ty * NTX + tx
                    pt = ps.tile([128, 512], f32, tag="pt", name=f"pt{t}")
                    for ch in range(2):
                        nc.tensor.matmul(
                            pt[:, 0:NFREE], in1_sb[:, t, ch, :], win_ap(ch, ty, tx),
                            start=(ch == 0), stop=(ch == 1),
                        )
                    finish(t, pt)
                    if ty == NTY - 1 and tx == 7:
                        # drain the last row's first half early (short tail)
                        store_rows(ty * NTX, ty * NTX + 8)
                if ty < NTY - 1:
                    store_rows(ty * NTX, (ty + 1) * NTX)
                else:
                    store_rows(ty * NTX + 8, (ty + 1) * NTX)

    nc.compile()
    return nc


def _prep_inputs(input1, input2):
    """Build per-core input maps (fp8, padded, tiled, c split on partitions)."""
    in_maps = []
    pad2 = np.pad(
        np.asarray(input2), ((0, 0), (0, 0), (OFF, OFF), (OFF, OFF))
    )  # [B, C, H+8, WP]
    a1 = np.asarray(input1)
    for core in range(NCORES):
        b, yh = core // 2, core % 2
        y0 = yh * YH
        i1 = a1[b, :, y0 : y0 + YH, :].reshape(2, 128, NTY, MT_Y, NTX, MT_X)
        # -> [p, ty, tx, ch, ly, lx] -> [128, NT, 2, 128]
        i1 = i1.transpose(1, 2, 4, 0, 3, 5).reshape(128, NT, 2, MT_Y * MT_X)
        p2 = pad2[b, :, y0 : y0 + ROWS, :].reshape(2, 128, ROWS, WP)
        i2c = p2.transpose(1, 0, 2, 3).astype(_F8)
        in_maps.append(
            {
                "in1t": np.ascontiguousarray(i1.astype(_F8)),
                "in2c": np.ascontiguousarray(i2c),
            }
        )
    return in_maps


def _extract(band):
    """band [128, NT, 144] f16 -> out_local [9, 9, 48, 128] f32.

    band[ly*8+lx, ty*16+tx, di*16 + lx + dj] = out[di, dj, ty*16+ly, tx*8+lx]
    """
    b6 = band.reshape(MT_Y, MT_X, NTY, NTX, P, NW_X).astype(np.float32)
    # [ly, lx, ty, tx, di, wxx] ; pick wxx = lx + dj
    sel = np.stack(
        [b6[:, lx, :, :, :, lx : lx + P] for lx in range(MT_X)], axis=1
    )  # [ly, lx, ty, tx, di, dj]
    out = sel.transpose(4, 5, 2, 0, 3, 1).reshape(P, P, YH, W)
    return out


def run(input1, input2, trace=False, **trace_kwargs):
    if "nc" not in _cached:
        _cached["nc"] = _build()
    nc = _cached["nc"]
    in_maps = _prep_inputs(input1, input2)
    res = run_bass_kernel_spmd(
        nc, in_maps, list(range(NCORES)), trace=trace, **trace_kwargs
    )
    out = np.empty((B, P, P, H, W), dtype=np.float32)
    for core in range(NCORES):
        b, yh = core // 2, core % 2
        band = res.results[core]["band"]
        out[b, :, :, yh * YH : (yh + 1) * YH, :] = _extract(band)
    return out, res


def kernel(input1, input2):
    out, _ = run(input1, input2, trace=False)
    return out


# revision 5
# speedup vs baseline: 1.1229x; 1.1229x over previous
"""IterSpatialCorrelationSampler (P=9, DP=1) Trainium2 Bass kernel.

out[b,i,j,y,x] = sum_c in1[b,c,y,x] * pad(in2)[b,c,y+i,x+j]   (pad=4 each side)

Strategy (v3):
  - 8 cores, each handles (b, yhalf): b = core//2, 48 rows of y.
  - TensorE Gram-band formulation with a 16y x 8x m-tile (128 output
    positions on PSUM partitions, p = ly*8+lx) against a 24y x 16x window
    of padded in2 (n = 384 free), contraction over c (2 accumulating
    k=128 matmuls).  PSUM tiles are bank PAIRS; one DVE/ACT copy per pair.
  - Compact stores: for partition p = (ly, lx) the 81 useful band values
    sit at band[p, ly*16 + (di*16 + lx + dj)].  Within one ly-group
    (8 partitions, same ly) the useful span [16ly, 16ly+144) is uniform,
    so a rectangular DMA  band[8ly:8ly+8, t0:t1, 16ly:16ly+144] ships
    only 144 of 384 band elements per position (1.77 MB/core vs 4.72).
    The store access patterns put the TILE dim outermost so descriptors
    spread over all 16 SDMA engines (partition-outer APs land on 8).
    2 chunks x 16 ly-groups = 32 stores, spread over the sync + gpsimd +
    scalar DMA queues (each dma_start costs ~0.65us of issuing-engine
    time, so count is kept low and off the copy engines where possible).
  - Inputs quantized host-side to fp8 e3m4 (rel err 1.8e-2, under the
    2e-2 bar); PSUM accumulates fp32; band ships as f16.
  - Loads are emitted in deadline order across the two HWDGE queues;
    ty0 runs a ch0-only pass over its first 8 tiles so the PE has work
    before the ch1 image chunk lands; 6 dummy matmuls during the load
    window ramp the PE clock (HAM).
"""

import numpy as np

import concourse.bass as bass
import concourse.bacc as bacc
import concourse.tile as tile
import concourse.mybir as mybir
from concourse.bass_utils import run_bass_kernel_spmd

# problem constants (hardcoded per contract)
B, C, H, W = 4, 256, 96, 128
P = 9
OFF = 4
NCORES = 8
YH = H // 2          # 48 rows per core
WP = W + 2 * OFF     # 136
ROWS = YH + 2 * OFF  # 56 rows of padded in2 per core
MT_Y, MT_X = 16, 8   # m-tile shape (16y x 8x = 128 partitions)
NW_Y, NW_X = MT_Y + P - 1, MT_X + P - 1   # 24 x 16 window
NTY, NTX = YH // MT_Y, W // MT_X          # 3 x 16 = 48 tiles
NT = NTY * NTX
NFREE = NW_Y * NW_X                       # 384
SPAN = P * NW_X                           # 144 elems shipped per position

_F8 = mybir.dt.np(mybir.dt.float8e3)   # ml_dtypes.float8_e3m4

_cached = {}


def _build():
    nc = bacc.Bacc(
        "TRN2",
        target_bir_lowering=False,
        debug=False,
        enable_asserts=False,
        num_devices=NCORES,
    )
    f16 = mybir.dt.float16
    f32 = mybir.dt.float32
    f8 = mybir.dt.float8e3

    in1_d = nc.dram_tensor("in1t", [128, NT, 2, MT_Y * MT_X], f8, kind="ExternalInput").ap()
    in2_d = nc.dram_tensor("in2c", [128, 2, ROWS, WP], f8, kind="ExternalInput").ap()
    # DRAM band laid out tile-major so store APs are [t, p, span]
    band_d = nc.dram_tensor("band", [NT, 128, SPAN], f16, kind="ExternalOutput").ap()

    with tile.TileContext(nc) as tc:
        with (
            tc.tile_pool(name="sb", bufs=1) as sb,
            tc.tile_pool(name="ps", bufs=4, space="PSUM") as ps,
        ):
            in2_sb = sb.tile([128, 2, ROWS, WP], f8, name="in2sb")
            in1_sb = sb.tile([128, NT, 2, MT_Y * MT_X], f8, name="in1sb")
            band = sb.tile([128, NT, NFREE], f16, name="band")

            # Loads in deadline order, alternating the two HWDGE queues.
            nc.sync.dma_start(out=in2_sb[:, 0, 0:24, :], in_=in2_d[:, 0, 0:24, :])
            nc.scalar.dma_start(out=in1_sb[:, 0:8, :, :], in_=in1_d[:, 0:8, :, :])
            nc.sync.dma_start(out=in2_sb[:, 1, 0:24, :], in_=in2_d[:, 1, 0:24, :])
            nc.scalar.dma_start(out=in1_sb[:, 8:16, :, :], in_=in1_d[:, 8:16, :, :])
            nc.sync.dma_start(out=in2_sb[:, 0, 24:40, :], in_=in2_d[:, 0, 24:40, :])
            nc.scalar.dma_start(out=in2_sb[:, 1, 24:40, :], in_=in2_d[:, 1, 24:40, :])
            nc.sync.dma_start(out=in1_sb[:, 16:32, :, :], in_=in1_d[:, 16:32, :, :])
            nc.scalar.dma_start(out=in2_sb[:, 0, 40:ROWS, :], in_=in2_d[:, 0, 40:ROWS, :])
            nc.sync.dma_start(out=in2_sb[:, 1, 40:ROWS, :], in_=in2_d[:, 1, 40:ROWS, :])
            nc.scalar.dma_start(out=in1_sb[:, 32:NT, :, :], in_=in1_d[:, 32:NT, :, :])

            # Warm up the PE while the first loads stream in (HAM clock
            # ramp 1.2 -> 2.4 GHz needs ~3.4us of sustained PE activity).
            wu = sb.tile([128, 512], f8, name="wu")
            nc.vector.memset(wu[:, :], 0.0)
            wpt = ps.tile([128, 2, 512], f32, tag="pt", name="wpt")
            for i in range(6):
                nc.tensor.matmul(
                    wpt[:, i % 2, :], wu[:, 0:128], wu[:, :], start=True, stop=True
                )

            def win_ap(ch, ty, tx):
                return in2_sb[
                    :, ch,
                    MT_Y * ty : MT_Y * ty + NW_Y,
                    MT_X * tx : MT_X * tx + NW_X,
                ]

            def copy_pair(tp, pt):
                # one PSUM->SBUF f16 copy per bank pair, alternating engines
                if tp % 2 == 0:
                    nc.vector.tensor_copy(
                        band[:, 2 * tp : 2 * tp + 2, :], pt[:, :, 0:NFREE]
                    )
                else:
                    nc.scalar.mul(
                        band[:, 2 * tp : 2 * tp + 2, :], pt[:, :, 0:NFREE], 1.0
                    )

            qrr = [0]

            def store_chunk(t0, t1):
                # 16 DMAs (one per ly-group).  Each 8-partition store maps
                # to only 2 SDMA engines (partition->port map), and ly 0..7
                # share one 8-engine half: interleave ly across the halves
                # so concurrent stores cover all 16 engines.
                for ly in [x for pair in zip(range(8), range(8, 16)) for x in pair]:
                    src = band[8 * ly : 8 * ly + 8, t0:t1, 16 * ly : 16 * ly + SPAN]
                    dst = band_d[t0:t1, 8 * ly : 8 * ly + 8, :].transpose([1, 0, 2])
                    eng = (nc.sync, nc.gpsimd, nc.scalar)[qrr[0] % 3]
                    qrr[0] += 1
                    eng.dma_start(out=dst, in_=src)

            def mm(pt, j, t, ch, ty, tx, start, stop):
                nc.tensor.matmul(
                    pt[:, j, 0:NFREE], in1_sb[:, t, ch, :], win_ap(ch, ty, tx),
                    start=start, stop=stop,
                )

            for ty in range(NTY):
                if ty == 0:
                    # ch0-only pass over tx 0..7 (ch1 image lands later)
                    pts = []
                    for tp in range(4):
                        pt = ps.tile([128, 2, 512], f32, tag="pt", name=f"pt0_{tp}")
                        pts.append(pt)
                        for j in range(2):
                            mm(pt, j, 2 * tp + j, 0, 0, 2 * tp + j, True, False)
                    for tp in range(4):
                        for j in range(2):
                            mm(pts[tp], j, 2 * tp + j, 1, 0, 2 * tp + j, False, True)
                        copy_pair(tp, pts[tp])
                    rng = range(4, 8)
                else:
                    rng = range(8)
                for tp in rng:
                    tpg = ty * 8 + tp
                    pt = ps.tile([128, 2, 512], f32, tag="pt", name=f"pt{tpg}")
                    for j in range(2):
                        t = 2 * tpg + j
                        tx = t - ty * NTX
                        for ch in range(2):
                            mm(pt, j, t, ch, ty, tx, ch == 0, ch == 1)
                    copy_pair(tpg, pt)
                    if tpg == 15:
                        store_chunk(0, 32)
            store_chunk(32, NT)

    nc.compile()
    return nc


def _prep_inputs(input1, input2):
    """Build per-core input maps (fp8, padded, tiled, c split on partitions)."""
    in_maps = []
    pad2 = np.pad(
        np.asarray(input2), ((0, 0), (0, 0), (OFF, OFF), (OFF, OFF))
    )  # [B, C, H+8, WP]
    a1 = np.asarray(input1)
    for core in range(NCORES):
        b, yh = core // 2, core % 2
        y0 = yh * YH
        i1 = a1[b, :, y0 : y0 + YH, :].reshape(2, 128, NTY, MT_Y, NTX, MT_X)
        # -> [p, ty, tx, ch, ly, lx] -> [128, NT, 2, 128]
        i1 = i1.transpose(1, 2, 4, 0, 3, 5).reshape(128, NT, 2, MT_Y * MT_X)
        p2 = pad2[b, :, y0 : y0 + ROWS, :].reshape(2, 128, ROWS, WP)
        i2c = p2.transpose(1, 0, 2, 3).astype(_F8)
        in_maps.append(
            {
                "in1t": np.ascontiguousarray(i1.astype(_F8)),
                "in2c": np.ascontiguousarray(i2c),
            }
        )
    return in_maps


def _extract(band):
    """band [NT, 128, 144] f16 -> out_local [9, 9, 48, 128] f32.

    band[ty*16+tx, ly*8+lx, di*16 + lx + dj] = out[di, dj, ty*16+ly, tx*8+lx]
    """
    b6 = band.reshape(NTY, NTX, MT_Y, MT_X, P, NW_X).astype(np.float32)
    # [ty, tx, ly, lx, di, wxx] ; pick wxx = lx + dj
    sel = np.stack(
        [b6[:, :, :, lx, :, lx : lx + P] for lx in range(MT_X)], axis=3
    )  # [ty, tx, ly, lx, di, dj]
    out = sel.transpose(4, 5, 0, 2, 1, 3).reshape(P, P, YH, W)
    return out


def run(input1, input2, trace=False, **trace_kwargs):
    if "nc" not in _cached:
        _cached["nc"] = _build()
    nc = _cached["nc"]
    in_maps = _prep_inputs(input1, input2)
    res = run_bass_kernel_spmd(
        nc, in_maps, list(range(NCORES)), trace=trace, **trace_kwargs
    )
    out = np.empty((B, P, P, H, W), dtype=np.float32)
    for core in range(NCORES):
        b, yh = core // 2, core % 2
        band = res.results[core]["band"]
        out[b, :, :, yh * YH : (yh + 1) * YH, :] = _extract(band)
    return out, res


def kernel(input1, input2):
    out, _ = run(input1, input2, trace=False)
    return out


# revision 6
# speedup vs baseline: 1.3910x; 1.2388x over previous
"""IterSpatialCorrelationSampler (P=9, DP=1) Trainium2 Bass kernel.

out[b,i,j,y,x] = sum_c in1[b,c,y,x] * pad(in2)[b,c,y+i,x+j]   (pad=4 each side)

Strategy (v3):
  - 8 cores, each handles (b, yhalf): b = core//2, 48 rows of y.
  - TensorE Gram-band formulation with a 16y x 8x m-tile (128 output
    positions on PSUM partitions, p = ly*8+lx) against a 24y x 16x window
    of padded in2 (n = 384 free), contraction over c (2 accumulating
    k=128 matmuls).  PSUM tiles are bank PAIRS; one DVE/ACT copy per pair.
  - Compact stores: for partition p = (ly, lx) the 81 useful band values
    sit at band[p, ly*16 + (di*16 + lx + dj)].  Within one ly-group
    (8 partitions, same ly) the useful span [16ly, 16ly+144) is uniform,
    so a rectangular DMA  band[8ly:8ly+8, t0:t1, 16ly:16ly+144] ships
    only 144 of 384 band elements per position (1.77 MB/core vs 4.72).
    The store access patterns put the TILE dim outermost so descriptors
    spread over all 16 SDMA engines (partition-outer APs land on 8).
    2 chunks x 16 ly-groups = 32 stores, spread over the sync + gpsimd +
    scalar DMA queues (each dma_start costs ~0.65us of issuing-engine
    time, so count is kept low and off the copy engines where possible).
  - Inputs quantized host-side to fp8 e3m4 (rel err 1.8e-2, under the
    2e-2 bar); PSUM accumulates fp32; band ships as f16.
  - Loads are emitted in deadline order across the two HWDGE queues;
    ty0 runs a ch0-only pass over its first 8 tiles so the PE has work
    before the ch1 image chunk lands; 6 dummy matmuls during the load
    window ramp the PE clock (HAM).
"""

import numpy as np

import concourse.bass as bass
import concourse.bacc as bacc
import concourse.tile as tile
import concourse.mybir as mybir
from concourse.bass_utils import run_bass_kernel_spmd

# problem constants (hardcoded per contract)
B, C, H, W = 4, 256, 96, 128
P = 9
OFF = 4
NCORES = 8
YH = H // 2          # 48 rows per core
WP = W + 2 * OFF     # 136
ROWS = YH + 2 * OFF  # 56 rows of padded in2 per core
MT_Y, MT_X = 16, 8   # m-tile shape (16y x 8x = 128 partitions)
NW_Y, NW_X = MT_Y + P - 1, MT_X + P - 1   # 24 x 16 window
NTY, NTX = YH // MT_Y, W // MT_X          # 3 x 16 = 48 tiles
NT = NTY * NTX
NFREE = NW_Y * NW_X                       # 384
SPAN = P * NW_X + MT_X + 8                # 160: union span of a 16-partition
                                          # group (two ly-rows) in the band

_F8 = mybir.dt.np(mybir.dt.float8e3)   # ml_dtypes.float8_e3m4

_cached = {}


def _build():
    nc = bacc.Bacc(
        "TRN2",
        target_bir_lowering=False,
        debug=False,
        enable_asserts=False,
        num_devices=NCORES,
    )
    f16 = mybir.dt.float16
    f32 = mybir.dt.float32
    f8 = mybir.dt.float8e3

    in1_d = nc.dram_tensor("in1t", [128, NT, 2, MT_Y * MT_X], f8, kind="ExternalInput").ap()
    in2_d = nc.dram_tensor("in2c", [128, 2, ROWS, WP], f8, kind="ExternalInput").ap()
    band_d = nc.dram_tensor("band", [128, NT, SPAN], f16, kind="ExternalOutput").ap()

    with tile.TileContext(nc) as tc:
        with (
            tc.tile_pool(name="sb", bufs=1) as sb,
            tc.tile_pool(name="ps", bufs=4, space="PSUM") as ps,
        ):
            in2_sb = sb.tile([128, 2, ROWS, WP], f8, name="in2sb")
            in1_sb = sb.tile([128, NT, 2, MT_Y * MT_X], f8, name="in1sb")
            band = sb.tile([128, NT, NFREE], f16, name="band")

            # Loads in deadline order, alternating the two HWDGE queues.
            nc.sync.dma_start(out=in2_sb[:, 0, 0:24, :], in_=in2_d[:, 0, 0:24, :])
            nc.scalar.dma_start(out=in1_sb[:, 0:8, :, :], in_=in1_d[:, 0:8, :, :])
            nc.sync.dma_start(out=in2_sb[:, 1, 0:24, :], in_=in2_d[:, 1, 0:24, :])
            nc.scalar.dma_start(out=in1_sb[:, 8:16, :, :], in_=in1_d[:, 8:16, :, :])
            nc.sync.dma_start(out=in2_sb[:, 0, 24:40, :], in_=in2_d[:, 0, 24:40, :])
            nc.scalar.dma_start(out=in2_sb[:, 1, 24:40, :], in_=in2_d[:, 1, 24:40, :])
            nc.sync.dma_start(out=in1_sb[:, 16:32, :, :], in_=in1_d[:, 16:32, :, :])
            nc.scalar.dma_start(out=in2_sb[:, 0, 40:ROWS, :], in_=in2_d[:, 0, 40:ROWS, :])
            nc.sync.dma_start(out=in2_sb[:, 1, 40:ROWS, :], in_=in2_d[:, 1, 40:ROWS, :])
            nc.scalar.dma_start(out=in1_sb[:, 32:NT, :, :], in_=in1_d[:, 32:NT, :, :])

            # Warm up the PE while the first loads stream in (HAM clock
            # ramp 1.2 -> 2.4 GHz needs ~3.4us of sustained PE activity).
            wu = sb.tile([128, 512], f8, name="wu")
            nc.vector.memset(wu[:, :], 0.0)
            wpt = ps.tile([128, 2, 512], f32, tag="pt", name="wpt")
            for i in range(6):
                nc.tensor.matmul(
                    wpt[:, i % 2, :], wu[:, 0:128], wu[:, :], start=True, stop=True
                )

            def win_ap(ch, ty, tx):
                return in2_sb[
                    :, ch,
                    MT_Y * ty : MT_Y * ty + NW_Y,
                    MT_X * tx : MT_X * tx + NW_X,
                ]

            def copy_pair(tp, pt):
                # one PSUM->SBUF f16 copy per bank pair, alternating engines
                if tp % 2 == 0:
                    nc.vector.tensor_copy(
                        band[:, 2 * tp : 2 * tp + 2, :], pt[:, :, 0:NFREE]
                    )
                else:
                    nc.scalar.mul(
                        band[:, 2 * tp : 2 * tp + 2, :], pt[:, :, 0:NFREE], 1.0
                    )

            qrr = [0]

            def store_chunk(t0, t1):
                # 8 DMAs, one per 16-partition group (two ly-rows sharing
                # the union span [32a, 32a+160)).  SDMA descriptors are
                # assigned per outer-dim row mod 16, so 16-row APs spread
                # across all 16 DMA engines (8-row APs pin to 8).
                for a in range(8):
                    src = band[16 * a : 16 * a + 16, t0:t1, 32 * a : 32 * a + SPAN]
                    dst = band_d[16 * a : 16 * a + 16, t0:t1, :]
                    eng = (nc.sync, nc.gpsimd, nc.scalar)[qrr[0] % 3]
                    qrr[0] += 1
                    eng.dma_start(out=dst, in_=src)

            def mm(pt, j, t, ch, ty, tx, start, stop):
                nc.tensor.matmul(
                    pt[:, j, 0:NFREE], in1_sb[:, t, ch, :], win_ap(ch, ty, tx),
                    start=start, stop=stop,
                )

            for ty in range(NTY):
                if ty == 0:
                    # ch0-only pass over tx 0..7 (ch1 image lands later)
                    pts = []
                    for tp in range(4):
                        pt = ps.tile([128, 2, 512], f32, tag="pt", name=f"pt0_{tp}")
                        pts.append(pt)
                        for j in range(2):
                            mm(pt, j, 2 * tp + j, 0, 0, 2 * tp + j, True, False)
                    for tp in range(4):
                        for j in range(2):
                            mm(pts[tp], j, 2 * tp + j, 1, 0, 2 * tp + j, False, True)
                        copy_pair(tp, pts[tp])
                    rng = range(4, 8)
                else:
                    rng = range(8)
                for tp in rng:
                    tpg = ty * 8 + tp
                    pt = ps.tile([128, 2, 512], f32, tag="pt", name=f"pt{tpg}")
                    for j in range(2):
                        t = 2 * tpg + j
                        tx = t - ty * NTX
                        for ch in range(2):
                            mm(pt, j, t, ch, ty, tx, ch == 0, ch == 1)
                    copy_pair(tpg, pt)
                    if tpg == 15:
                        store_chunk(0, 32)
            store_chunk(32, NT)

    nc.compile()
    return nc


def _prep_inputs(input1, input2):
    """Build per-core input maps (fp8, padded, tiled, c split on partitions)."""
    in_maps = []
    pad2 = np.pad(
        np.asarray(input2), ((0, 0), (0, 0), (OFF, OFF), (OFF, OFF))
    )  # [B, C, H+8, WP]
    a1 = np.asarray(input1)
    for core in range(NCORES):
        b, yh = core // 2, core % 2
        y0 = yh * YH
        i1 = a1[b, :, y0 : y0 + YH, :].reshape(2, 128, NTY, MT_Y, NTX, MT_X)
        # -> [p, ty, tx, ch, ly, lx] -> [128, NT, 2, 128]
        i1 = i1.transpose(1, 2, 4, 0, 3, 5).reshape(128, NT, 2, MT_Y * MT_X)
        p2 = pad2[b, :, y0 : y0 + ROWS, :].reshape(2, 128, ROWS, WP)
        i2c = p2.transpose(1, 0, 2, 3).astype(_F8)
        in_maps.append(
            {
                "in1t": np.ascontiguousarray(i1.astype(_F8)),
                "in2c": np.ascontiguousarray(i2c),
            }
        )
    return in_maps


def _extract(band):
    """band [128, NT, 160] f16 -> out_local [9, 9, 48, 128] f32.

    For partition p = ly*8+lx (group a = p//16, r = p%16):
      band[p, t, 16*(r//8) + di*16 + lx + dj] = out[di, dj, ty*16+ly, tx*8+lx]
    """
    b = band.reshape(MT_Y, MT_X, NTY, NTX, SPAN).astype(np.float32)
    # [ly, lx, ty, tx, e];  e = 16*(ly%2) + 16*di + lx + dj
    sel = np.empty((MT_Y, MT_X, NTY, NTX, P, P), dtype=np.float32)
    for ly in range(MT_Y):
        for lx in range(MT_X):
            base = 16 * (ly % 2) + lx
            v = b[ly, lx, :, :, base : base + 16 * (P - 1) + P]
            sel[ly, lx] = v.reshape(NTY, NTX, -1)[:, :, : 16 * (P - 1) + P : 1][
                :, :, [16 * di + dj for di in range(P) for dj in range(P)]
            ].reshape(NTY, NTX, P, P)
    out = sel.transpose(4, 5, 2, 0, 3, 1).reshape(P, P, YH, W)
    return out


def run(input1, input2, trace=False, **trace_kwargs):
    if "nc" not in _cached:
        _cached["nc"] = _build()
    nc = _cached["nc"]
    in_maps = _prep_inputs(input1, input2)
    res = run_bass_kernel_spmd(
        nc, in_maps, list(range(NCORES)), trace=trace, **trace_kwargs
    )
    out = np.empty((B, P, P, H, W), dtype=np.float32)
    for core in range(NCORES):
        b, yh = core // 2, core % 2
        band = res.results[core]["band"]
        out[b, :, :, yh * YH : (yh + 1) * YH, :] = _extract(band)
    return out, res


def kernel(input1, input2):
    out, _ = run(input1, input2, trace=False)
    return out
